# revision 32
# baseline (speedup 1.0000x reference)
"""Self-contained TRN2 kernel for nn_FLASH_ShareA_FFConvM_FlashAttn.

kernel(**inputs) takes the full (unsharded) inputs from setup_inputs() and
returns the full (B, N, D) float32 output. Internally: data-parallel over the
batch — one batch sample per NeuronCore, 8 cores, no collectives.

v2: all heavy matmuls in fp8 DoubleRow (paired k-tiles / paired conv taps),
depthwise convs fully on the PE as paired diagonal matmuls, attention weights
pre-scaled by 2^30 to stay in fp8 range, deferred output LayerNorm, and the
zspill round-trip replaced by an SBUF-resident transposed buffer.
"""
import sys

if "/opt/trn_rl_repo" not in sys.path:
    sys.path.insert(0, "/opt/trn_rl_repo")

import numpy as np
import ml_dtypes
import concourse.bass as bass
import concourse.bacc as bacc
import concourse.mybir as mybir
import concourse.tile as tile
from concourse import bass_utils
from concourse.ap import AP

F32 = mybir.dt.float32
BF16 = mybir.dt.bfloat16
FP8 = mybir.dt.float8e4
AF = mybir.ActivationFunctionType
OP = mybir.AluOpType
DR = mybir.MatmulPerfMode.DoubleRow

N, D, H, QK, G = 4096, 512, 2048, 128, 256
NG = N // G
NT = N // 128
KTAPS = 17
PAD = 8
NPADBUF = N + 2 * PAD  # fp8/bf16 padded conv input length (max tap read = N+15)
E2 = 2 * D
EPS = 1e-5
NCH = 21  # conv channel tiles: hid 0..15, out 16..19, qk 20
# conv tap pairs with stride-4 spacing (DR rows must be >=4 fp8 elements apart).
# Taps 8,12,9,13,16 run on the DVE instead (frees PE passes during P3/P5).
PAIRS = [(0, 4), (1, 5), (2, 6), (3, 7), (10, 14), (11, 15)]
DVETAPS = [8, 12, 9, 13, 16]
ASCALE = float(2 ** 30)      # attention-weight scale kept inside psum
RELUSC = float(2 ** 15)      # sqrt(ASCALE), applied before squaring
GRP = 16                     # P4 deferred-LN batch size (iterations)


def _pair_ap(t, off, n):
    """Overlapping [128, 2, n] moving AP: row j reads t[:, off+4j : off+4j+n]."""
    base = t[:, 0:1]
    return AP(base.tensor, base.offset + off, [list(base.ap[0]), [4, 2], [1, n]])


def _emit_conv(nc, pool, dvp, dg, wv, h8t, hb, acc):
    """acc = h + conv(h): 6 DR tap pairs + tap16 on PE; 4 taps + identity on DVE."""
    # DVE partial: accd = h + sum_{k in DVETAPS} w_k * h_shift_k   (full width)
    accd = dvp.tile([128, N], BF16, tag="accd")
    tmp = dvp.tile([128, N], BF16, tag="dvtmp")
    for j, k in enumerate(DVETAPS):
        nc.vector.tensor_scalar(out=tmp[:, :], in0=hb[:, k:k + N],
                                scalar1=wv[:, j:j + 1], scalar2=None, op0=OP.mult)
        if j == 0:
            nc.vector.tensor_tensor(out=accd[:, :], in0=tmp[:, :],
                                    in1=hb[:, PAD:PAD + N], op=OP.add)
        else:
            nc.vector.tensor_tensor(out=accd[:, :], in0=tmp[:, :],
                                    in1=accd[:, :], op=OP.add)
    for c in range(8):
        cb = c * 512
        cp = pool.tile([128, 512], F32, tag="convps")
        for pr in range(6):
            nc.tensor.matmul(cp[:, :], dg[:, pr, :, :], _pair_ap(h8t, PAIRS[pr][0] + cb, 512),
                             start=(pr == 0), stop=(pr == 5), perf_mode=DR)
        evac = dvp.tile([128, 512], BF16, tag="cevac")
        nc.scalar.activation(evac[:, :], cp[:, :], AF.Copy)
        nc.vector.tensor_tensor(out=acc[:, cb:cb + 512], in0=evac[:, :],
                                in1=accd[:, cb:cb + 512], op=OP.add)


def _emit(nc, tc, x, wh8, wqk8, wo8, bh, bqk, bo, gb, diag, dwv, out, spill):
    from contextlib import ExitStack
    es = ExitStack()
    consts = es.enter_context(tc.tile_pool(name="consts", bufs=1))
    wh_sb = consts.tile([128, 2, 2, H], FP8)
    nc.sync.dma_start(wh_sb[:, :, :, :], wh8.ap())
    wqk_sb = consts.tile([128, 2, 2, QK], FP8)
    nc.sync.dma_start(wqk_sb[:, :, :, :], wqk8.ap())
    wo_sb = consts.tile([128, 4, 2, D], FP8)
    nc.sync.dma_start(wo_sb[:, :, :, :], wo8.ap())
    bh_sb = consts.tile([128, 16], F32)
    nc.sync.dma_start(bh_sb[:, :], bh.ap())
    bqk_sb = consts.tile([128, 1], F32)
    nc.sync.dma_start(bqk_sb[:, :], bqk.ap())
    bo_sb = consts.tile([128, 4], F32)
    nc.sync.dma_start(bo_sb[:, :], bo.ap())
    gb_sb = consts.tile([128, 8], F32)
    nc.sync.dma_start(gb_sb[:, :], gb.ap())
    dwv_sb = consts.tile([128, NCH, 5], F32)
    nc.sync.dma_start(dwv_sb[:, :, :], dwv.ap())
    eps_sb = consts.tile([128, 1], F32)
    nc.vector.memset(eps_sb[:, :], EPS)

    outer = es.enter_context(tc.tile_pool(name="outer", bufs=1))
    attnT8 = outer.tile([128, NG, 2, G], FP8)
    lq_sb = outer.tile([128, N], BF16)
    lk_str = outer.tile([128, NT, 128], BF16)
    linkv_sb = outer.tile([128, E2], BF16)
    linku_sb = outer.tile([128, E2], BF16)
    sums = outer.tile([128, 32], F32)
    sumsq = outer.tile([128, 32], F32)
    spill_v = spill.ap().rearrange("(tt p) (q c4) -> p tt q c4", p=128, c4=512)
    xs_v = x.ap().rearrange("(c p) d -> p c d", p=128)

    es2 = ExitStack()
    zpool = es2.enter_context(tc.tile_pool(name="zpool", bufs=1))
    qq_sb = zpool.tile([128, N], BF16)
    qkk_sb = zpool.tile([128, N], BF16)
    zT8 = []
    for c in range(8):
        zT8c = zpool.tile([128, 4, 512], FP8, tag=f"zT8_{c}")
        zT8.append(zT8c)

    # ---------------- P0: token-shifted LayerNorm -> zT8 chunks ----------------
    # x loads batched 4 token-tiles per DMA; shifted first-half channels loaded
    # separately with a one-row offset.
    xs_v = x.ap().rearrange("(c p) d -> p c d", p=128)
    with tc.tile_pool(name="p0", bufs=4) as p0, \
         tc.tile_pool(name="p0z", bufs=4) as p0z, \
         tc.tile_pool(name="p0s", bufs=8) as p0s:
        for cch in (0, 1, 2, 3, 4, 5, 6, 7):
            x4 = p0.tile([128, 4, D], F32, tag="x4")
            t0 = cch * 512
            if cch == 0:
                nc.vector.memset(x4[0:1, 0, 0:D // 2], 0.0)
                nc.gpsimd.dma_start(x4[1:128, 0, 0:D // 2], x[0:127, 0:D // 2])
                for j in range(1, 4):
                    nc.gpsimd.dma_start(x4[:, j, 0:D // 2],
                                        x[t0 + j * 128 - 1:t0 + j * 128 + 127, 0:D // 2])
            else:
                nc.gpsimd.dma_start(
                    x4[:, :, 0:D // 2],
                    x.ap()[t0 - 1:t0 + 511, 0:D // 2].rearrange("(j p) d -> p j d", p=128))
            nc.gpsimd.dma_start(x4[:, :, D // 2:D],
                                xs_v[:, 4 * cch:4 * cch + 4, D // 2:D])
            ztc = p0z.tile([128, 4, 512], BF16, tag="ztc")
            for j in range(4):
                st6 = p0s.tile([128, 6], F32, tag="st6")
                nc.vector.bn_stats(st6[:, :], x4[:, j, :])
                mv = p0s.tile([128, 2], F32, tag="mv")
                nc.vector.bn_aggr(mv[:, :], st6[:, :])
                rstd = p0s.tile([128, 1], F32, tag="rstd")
                nc.scalar.activation(rstd[:, :], mv[:, 1:2], AF.Sqrt, bias=eps_sb[:, :], scale=1.0)
                nc.vector.reciprocal(rstd[:, :], rstd[:, :])
                nmu = p0s.tile([128, 1], F32, tag="nmu")
                nc.vector.tensor_scalar(out=nmu[:, :], in0=mv[:, 0:1], scalar1=rstd[:, :],
                                        scalar2=-1.0, op0=OP.mult, op1=OP.mult)
                zt = p0.tile([128, D], BF16, tag="zt")
                nc.scalar.activation(zt[:, :], x4[:, j, :], AF.Identity,
                                     bias=nmu[:, :], scale=rstd[:, :])
                eng = nc.sync if j % 2 == 0 else nc.scalar
                eng.dma_start_transpose(ztc[:, :, j * 128:j * 128 + 128], zt[:, :])
                nc.scalar.activation(zT8[cch][:, :, j * 128:j * 128 + 128],
                                     ztc[:, :, j * 128:j * 128 + 128], AF.Copy)

    # ---------------- P1+P3: qk path interleaved with hidden FFConvM ----------------
    with tc.tile_pool(name="p1", bufs=1) as p1, \
         tc.tile_pool(name="p3", bufs=2) as p3, \
         tc.tile_pool(name="p3d", bufs=3) as p3d, \
         tc.tile_pool(name="p1s", bufs=3) as p1s, \
         tc.tile_pool(name="p3p", bufs=3, space="PSUM") as p3p, \
         tc.tile_pool(name="p3cp", bufs=3, space="PSUM") as p3cp, \
         tc.tile_pool(name="p1sp", bufs=1, space="PSUM") as p1sp, \
         tc.tile_pool(name="p3lin", bufs=1, space="PSUM") as p3lin:
        state = {"strips4": None}

        def produce_start(hc):
            dg = p3d.tile([128, 7, 2, 128], FP8, tag="dg")
            nc.gpsimd.dma_start(dg[:, :, :, :], diag.ap()[:, hc, :, :, :])
            hb = p3.tile([128, NPADBUF], BF16, tag="hpad")
            nc.vector.memset(hb[:, 0:PAD], 0.0)
            nc.vector.memset(hb[:, PAD + N:], 0.0)
            return dg, hb

        def produce_chunk(hc, hb, c):
            ps = p3p.tile([128, 512], F32, tag="hps")
            for pr in range(2):
                nc.tensor.matmul(ps[:, :], wh_sb[:, pr, :, hc * 128:(hc + 1) * 128],
                                 zT8[c][:, 2 * pr:2 * pr + 2, :],
                                 start=(pr == 0), stop=(pr == 1), perf_mode=DR)
            nc.scalar.activation(hb[:, PAD + c * 512:PAD + (c + 1) * 512], ps[:, :],
                                 AF.Silu, bias=bh_sb[:, hc:hc + 1], scale=1.0)

        def produce_end(hb):
            h8 = p3.tile([128, NPADBUF], FP8, tag="h8pad")
            nc.scalar.activation(h8[:, :], hb[:, :], AF.Copy)
            return h8

        def produce(hc):
            dg, hb = produce_start(hc)
            for c in range(8):
                produce_chunk(hc, hb, c)
            return dg, hb, produce_end(hb)

        def convpost(hc, dg, hb, h8):
            if hc % 4 == 0:
                s4_new = p3q.tile([128, NT, 4, 128], BF16, tag="strips4")
                state["strips4"] = s4_new
            strips4 = state["strips4"]
            acc = p3.tile([128, N], BF16, tag="acc")
            _emit_conv(nc, p3cp, p3v, dg, dwv_sb[:, hc, :], h8, hb, acc)
            nc.sync.dma_start_transpose(strips4[:, :, hc % 4, :], acc[:, :])
            if hc % 4 == 3:
                q = hc // 4
                nc.gpsimd.dma_start(spill_v[:, :, q, :], strips4[:, :, :, :])
                kvp = p3lin.tile([128, 512], F32, tag="kvps")
                for tt in range(NT):
                    nc.tensor.matmul(
                        kvp[:, :], lk_str[:, tt, :],
                        strips4[:, tt, :, :].rearrange("p a c -> p (a c)"),
                        start=(tt == 0), stop=(tt == NT - 1))
                dst = linkv_sb if q < 2 else linku_sb
                nc.scalar.activation(dst[:, (q % 2) * 512:(q % 2) * 512 + 512],
                                     kvp[:, :], AF.Copy)

        # Front: chunk-major over {qk, hc0, hc1} so the PE consumes zT8 chunks
        # the moment P0 produces them (no head-of-line blocking on later chunks).
        dgq = p3d.tile([128, 7, 2, 128], FP8, tag="dg")
        nc.gpsimd.dma_start(dgq[:, :, :, :], diag.ap()[:, 20, :, :, :])
        qkp = p1.tile([128, NPADBUF], BF16, tag="qkpad")
        nc.vector.memset(qkp[:, 0:PAD], 0.0)
        nc.vector.memset(qkp[:, PAD + N:], 0.0)
        q8p = p1.tile([128, NPADBUF], FP8, tag="qk8pad")
        dg0, hb0 = produce_start(0)
        dg1, hb1 = produce_start(1)
        for c in range(8):
            ps = p3p.tile([128, 512], F32, tag="hps")
            for pr in range(2):
                nc.tensor.matmul(ps[:, :], wqk_sb[:, pr, :, :],
                                 zT8[c][:, 2 * pr:2 * pr + 2, :],
                                 start=(pr == 0), stop=(pr == 1), perf_mode=DR)
            nc.scalar.activation(qkp[:, PAD + c * 512:PAD + (c + 1) * 512], ps[:, :],
                                 AF.Silu, bias=bqk_sb[:, :], scale=1.0)
            produce_chunk(0, hb0, c)
            produce_chunk(1, hb1, c)
        es4 = ExitStack()
        p3q = es4.enter_context(tc.tile_pool(name="p3q", bufs=1))
        p3v = es4.enter_context(tc.tile_pool(name="p3v", bufs=2))
        nc.scalar.activation(q8p[:, :], qkp[:, :], AF.Copy)
        h80 = produce_end(hb0)
        h81 = produce_end(hb1)
        qkc = p3.tile([128, N], BF16, tag="acc")
        _emit_conv(nc, p3cp, p3v, dgq, dwv_sb[:, 20, :], q8p, qkp, qkc)
        lkk = p3.tile([128, N], BF16, tag="acc")
        for i, dst in ((0, qq_sb), (1, lq_sb), (2, qkk_sb), (3, lkk)):
            nc.vector.tensor_scalar(out=dst[:, :], in0=qkc[:, :], scalar1=gb_sb[:, i:i + 1],
                                    scalar2=gb_sb[:, 4 + i:5 + i], op0=OP.mult, op1=OP.add)
        nc.sync.dma_start_transpose(lk_str[:, :, :], lkk[:, :])
        convpost(0, dg0, hb0, h80)
        for g in range(NG):
            for jh in range(2):
                sp = p1sp.tile([128, G], F32, tag="simps")
                nc.tensor.matmul(sp[:, :],
                                 qkk_sb[:, g * G + jh * 128: g * G + jh * 128 + 128],
                                 qq_sb[:, g * G:(g + 1) * G],
                                 start=True, stop=True)
                rel = p1s.tile([128, G], BF16, tag="rel")
                nc.scalar.activation(rel[:, :], sp[:, :], AF.Relu, scale=RELUSC)
                nc.scalar.activation(attnT8[:, g, jh, :], rel[:, :], AF.Square)

        prev = (1, dg1, hb1, h81)
        for hc in range(2, 16):
            pr = produce(hc)
            convpost(*prev)
            prev = (hc, *pr)
        convpost(*prev)
        es4.close()

    es2.close()  # frees zT8 chunks before the P4/P5 pools open

    # ---------------- P4: attention apply + gating (deferred LN) ----------------
    with tc.tile_pool(name="mid", bufs=1) as mid:
        zoT8 = mid.tile([128, 8, N], FP8)
        vo_big = mid.tile([128, NT, 4, 128], BF16)
        with tc.tile_pool(name="p4", bufs=2) as p4, \
             tc.tile_pool(name="p4g", bufs=GRP + 2) as p4g, \
             tc.tile_pool(name="p4s", bufs=3) as p4s, \
             tc.tile_pool(name="p4p", bufs=2, space="PSUM") as p4p:
            govu = []   # (go, vt?, ...) per pending it in current group
            for g in range(NG):
                vt, ut = [], []
                for jh in range(2):
                    vtj = p4.tile([128, E2], BF16, tag=f"vg{jh}")
                    nc.gpsimd.dma_start(vtj[:, :], spill[g * G + jh * 128: g * G + jh * 128 + 128, 0:E2])
                    utj = p4.tile([128, E2], BF16, tag=f"ug{jh}")
                    nc.gpsimd.dma_start(utj[:, :], spill[g * G + jh * 128: g * G + jh * 128 + 128, E2:H])
                    vt.append(vtj)
                    ut.append(utj)
                vt8 = p4.tile([128, 2, 2, 512], FP8, tag="vt8")
                ut8 = p4.tile([128, 2, 2, 512], FP8, tag="ut8")
                for jh in range(2):
                    for e in range(2):
                        nc.vector.tensor_copy(vt8[:, e, jh, :], vt[jh][:, e * 512:(e + 1) * 512])
                        nc.scalar.activation(ut8[:, e, jh, :], ut[jh][:, e * 512:(e + 1) * 512],
                                             AF.Copy)
                for it in range(2):
                    idx = g * 2 + it
                    islice = slice(g * G + it * 128, g * G + it * 128 + 128)
                    avp = p4p.tile([128, E2], F32, tag="avps")
                    for e in range(2):
                        nc.tensor.matmul(avp[:, e * 512:(e + 1) * 512],
                                         attnT8[:, g, :, it * 128:it * 128 + 128],
                                         vt8[:, e, :, :],
                                         start=True, stop=False, perf_mode=DR)
                        nc.tensor.matmul(avp[:, e * 512:(e + 1) * 512],
                                         lq_sb[:, islice], linkv_sb[:, e * 512:(e + 1) * 512],
                                         start=False, stop=True)
                    t1 = p4s.tile([128, E2], BF16, tag="t1")
                    nc.vector.scalar_tensor_tensor(out=t1[:, :], in0=avp[:, :],
                                                   scalar=1.0 / ASCALE, in1=ut[it][:, :],
                                                   op0=OP.mult, op1=OP.mult)
                    sg = p4s.tile([128, E2], BF16, tag="sg")
                    nc.scalar.activation(sg[:, :], t1[:, :], AF.Sigmoid)
                    aup = p4p.tile([128, E2], F32, tag="avps")
                    for e in range(2):
                        nc.tensor.matmul(aup[:, e * 512:(e + 1) * 512],
                                         attnT8[:, g, :, it * 128:it * 128 + 128],
                                         ut8[:, e, :, :],
                                         start=True, stop=False, perf_mode=DR)
                        nc.tensor.matmul(aup[:, e * 512:(e + 1) * 512],
                                         lq_sb[:, islice], linku_sb[:, e * 512:(e + 1) * 512],
                                         start=False, stop=True)
                    t2 = p4s.tile([128, E2], BF16, tag="t2")
                    nc.vector.scalar_tensor_tensor(out=t2[:, :], in0=aup[:, :],
                                                   scalar=1.0 / ASCALE, in1=vt[it][:, :],
                                                   op0=OP.mult, op1=OP.mult)
                    go = p4g.tile([128, E2], BF16, tag="go")
                    nc.vector.scalar_tensor_tensor(out=go[:, :], in0=t2[:, :], scalar=1.0,
                                                   in1=sg[:, :], op0=OP.mult, op1=OP.mult,
                                                   accum_out=sums[:, idx:idx + 1])
                    jnk = p4s.tile([128, E2], BF16, tag="jnk")
                    nc.scalar.activation(jnk[:, :], go[:, :], AF.Square,
                                         accum_out=sumsq[:, idx:idx + 1])
                    govu.append(go)
                    if len(govu) == GRP:
                        _p4_norm(nc, tc, p4s, govu, sums, sumsq, eps_sb, zoT8,
                                 idx - GRP + 1)
                        govu = []

        # ---------------- P5: output FFConvM ----------------
        with tc.tile_pool(name="p5", bufs=2) as p5, \
             tc.tile_pool(name="p5d", bufs=2) as p5d, \
             tc.tile_pool(name="p5v", bufs=2) as p5v, \
             tc.tile_pool(name="p5p", bufs=2, space="PSUM") as p5p, \
             tc.tile_pool(name="p5cp", bufs=3, space="PSUM") as p5cp:
            def produce5(oc):
                dg = p5d.tile([128, 7, 2, 128], FP8, tag="dg5")
                nc.gpsimd.dma_start(dg[:, :, :, :], diag.ap()[:, 16 + oc, :, :, :])
                hb = p5.tile([128, NPADBUF], BF16, tag="hpad5")
                nc.vector.memset(hb[:, 0:PAD], 0.0)
                nc.vector.memset(hb[:, PAD + N:], 0.0)
                for c in range(8):
                    ps = p5p.tile([128, 512], F32, tag="ops")
                    for pr in range(4):
                        nc.tensor.matmul(ps[:, :], wo_sb[:, pr, :, oc * 128:(oc + 1) * 128],
                                         zoT8[:, 2 * pr:2 * pr + 2, c * 512:(c + 1) * 512],
                                         start=(pr == 0), stop=(pr == 3), perf_mode=DR)
                    nc.scalar.activation(hb[:, PAD + c * 512:PAD + (c + 1) * 512], ps[:, :],
                                         AF.Silu, bias=bo_sb[:, oc:oc + 1], scale=1.0)
                h8 = p5.tile([128, NPADBUF], FP8, tag="h85")
                nc.scalar.activation(h8[:, :], hb[:, :], AF.Copy)
                return dg, hb, h8

            def convpost5(oc, dg, hb, h8):
                acc = p5.tile([128, N], BF16, tag="acc5")
                _emit_conv(nc, p5cp, p5v, dg, dwv_sb[:, 16 + oc, :], h8, hb, acc)
                nc.sync.dma_start_transpose(vo_big[:, :, oc, :], acc[:, :])

            prev = None
            for oc in range(4):
                pr = produce5(oc)
                if prev is not None:
                    convpost5(*prev)
                prev = (oc, *pr)
            convpost5(*prev)

        # ---------------- P6: residual (4 token-tiles per iteration) ----------------
        out_v = out.ap().rearrange("(c p) d -> p c d", p=128)
        with tc.tile_pool(name="p6", bufs=3) as p6:
            for cch in range(8):
                xt = p6.tile([128, 4, D], F32, tag="xt6")
                nc.gpsimd.dma_start(xt[:, :, :], xs_v[:, 4 * cch:4 * cch + 4, :])
                of = p6.tile([128, 4, D], F32, tag="of")
                nc.vector.tensor_tensor(
                    out=of[:, :, :].rearrange("p a c -> p (a c)"),
                    in0=xt[:, :, :].rearrange("p a c -> p (a c)"),
                    in1=vo_big[:, 4 * cch:4 * cch + 4, :, :].rearrange("p a b c -> p (a b c)"),
                    op=OP.add)
                nc.gpsimd.dma_start(out_v[:, 4 * cch:4 * cch + 4, :], of[:, :, :])
    es.close()


def _p4_norm(nc, tc, pool, gos, sums, sumsq, eps_sb, zoT8, idx0):
    """Deferred LayerNorm for GRP gating tiles: batched stats then per-tile
    normalize + transpose + fp8 convert."""
    n = len(gos)
    sl = slice(idx0, idx0 + n)
    mean = pool.tile([128, n], F32, tag="mean")
    nc.vector.tensor_scalar(out=mean[:, :], in0=sums[:, sl], scalar1=1.0 / E2,
                            scalar2=None, op0=OP.mult)
    msq = pool.tile([128, n], F32, tag="msq")
    nc.vector.tensor_tensor(out=msq[:, :], in0=mean[:, :], in1=mean[:, :], op=OP.mult)
    var = pool.tile([128, n], F32, tag="var")
    nc.vector.scalar_tensor_tensor(out=var[:, :], in0=sumsq[:, sl], scalar=1.0 / E2,
                                   in1=msq[:, :], op0=OP.mult, op1=OP.subtract)
    rstd = pool.tile([128, n], F32, tag="rstdn")
    nc.scalar.activation(rstd[:, :], var[:, :], AF.Sqrt, bias=eps_sb[:, :], scale=1.0)
    nc.vector.reciprocal(rstd[:, :], rstd[:, :])
    nmu = pool.tile([128, n], F32, tag="nmun")
    nc.vector.tensor_tensor(out=nmu[:, :], in0=mean[:, :], in1=rstd[:, :], op=OP.mult)
    nc.vector.tensor_scalar(out=nmu[:, :], in0=nmu[:, :], scalar1=-1.0,
                            scalar2=None, op0=OP.mult)
    for j, go in enumerate(gos):
        tti = idx0 + j
        zo = pool.tile([128, E2], BF16, tag="zon")
        nc.vector.tensor_scalar(out=zo[:, :], in0=go[:, :], scalar1=rstd[:, j:j + 1],
                                scalar2=nmu[:, j:j + 1], op0=OP.mult, op1=OP.add)
        zot = pool.tile([128, 8, 128], BF16, tag="zot")
        (nc.sync if j % 2 == 0 else nc.scalar).dma_start_transpose(zot[:, :, :], zo[:, :])
        if j % 2 == 0:
            nc.scalar.activation(zoT8[:, :, tti * 128:(tti + 1) * 128], zot[:, :, :], AF.Copy)
        else:
            nc.vector.tensor_copy(zoT8[:, :, tti * 128:(tti + 1) * 128], zot[:, :, :])


def _build_nc():
    nc = bacc.Bacc("TRN2", target_bir_lowering=False, debug=False)
    x = nc.dram_tensor("x", [N, D], F32, kind="ExternalInput")
    wh8 = nc.dram_tensor("wh8", [128, 2, 2, H], FP8, kind="ExternalInput")
    wqk8 = nc.dram_tensor("wqk8", [128, 2, 2, QK], FP8, kind="ExternalInput")
    wo8 = nc.dram_tensor("wo8", [128, 4, 2, D], FP8, kind="ExternalInput")
    bh = nc.dram_tensor("bh", [128, 16], F32, kind="ExternalInput")
    bqk = nc.dram_tensor("bqk", [128, 1], F32, kind="ExternalInput")
    bo = nc.dram_tensor("bo", [128, 4], F32, kind="ExternalInput")
    gb = nc.dram_tensor("gb", [128, 8], F32, kind="ExternalInput")
    diag = nc.dram_tensor("diag", [128, NCH, 7, 2, 128], FP8, kind="ExternalInput")
    dwv = nc.dram_tensor("dwv", [128, NCH, 5], F32, kind="ExternalInput")
    out = nc.dram_tensor("out", [N, D], F32, kind="ExternalOutput")
    spill = nc.dram_tensor("spill", [N, H], BF16)
    with tile.TileContext(nc) as tc:
        _emit(nc, tc, x, wh8, wqk8, wo8, bh, bqk, bo, gb, diag, dwv, out, spill)
    nc.compile()
    return nc


def prep_inputs(inputs):
    f32 = np.float32
    fp8 = ml_dtypes.float8_e4m3
    W_h = np.asarray(inputs["W_h"], f32)
    W_qk = np.asarray(inputs["W_qk"], f32)
    W_o = np.asarray(inputs["W_o"], f32)
    whp = np.asarray(inputs["ln_h_g"], f32)[:, None] * W_h
    bhp = np.asarray(inputs["ln_h_b"], f32) @ W_h + np.asarray(inputs["b_h"], f32)
    wqkp = np.asarray(inputs["ln_qk_g"], f32)[:, None] * W_qk
    bqkp = np.asarray(inputs["ln_qk_b"], f32) @ W_qk + np.asarray(inputs["b_qk"], f32)
    wop = np.asarray(inputs["ln_o_g"], f32)[:, None] * W_o
    bop = np.asarray(inputs["ln_o_b"], f32) @ W_o + np.asarray(inputs["b_o"], f32)
    gamma = np.asarray(inputs["gamma"], f32).copy()
    beta = np.asarray(inputs["beta"], f32).copy()
    gamma[0] /= G
    beta[0] /= G
    gamma[1] *= ASCALE
    beta[1] *= ASCALE
    gamma[3] /= N
    beta[3] /= N

    def lhsT8(w, kt):
        # [din, dout] -> [128, kt/2 pairs, 2, dout] fp8
        t = w.reshape(kt, 128, -1).transpose(1, 0, 2)  # [128, kt, dout]
        return np.ascontiguousarray(
            t.reshape(128, kt // 2, 2, t.shape[-1])).astype(fp8)

    def chan(v, ntiles):
        return np.ascontiguousarray(v.reshape(ntiles, 128).T).astype(f32)

    # diagonal conv stationaries: [128, NCH, 9, 2, 128] fp8
    dw_h = np.asarray(inputs["dw_h"], f32)
    dw_o = np.asarray(inputs["dw_o"], f32)
    dw_qk = np.asarray(inputs["dw_qk"], f32)
    diag = np.zeros((128, NCH, 7, 2, 128), f32)
    dwv = np.zeros((128, NCH, 5), f32)
    rng128 = np.arange(128)
    for ct in range(NCH):
        if ct < 16:
            wsrc = dw_h[:, ct * 128:(ct + 1) * 128]
        elif ct < 20:
            wsrc = dw_o[:, (ct - 16) * 128:(ct - 15) * 128]
        else:
            wsrc = dw_qk
        for pr, (k0, k1) in enumerate(PAIRS):
            diag[rng128, ct, pr, 0, rng128] = wsrc[k0]
            diag[rng128, ct, pr, 1, rng128] = wsrc[k1]
        for j, k in enumerate(DVETAPS):
            dwv[:, ct, j] = wsrc[k]
    return {
        "wh8": lhsT8(whp, 4), "wqk8": lhsT8(wqkp, 4), "wo8": lhsT8(wop, 8),
        "bh": chan(bhp, 16), "bqk": chan(bqkp, 1), "bo": chan(bop, 4),
        "gb": np.concatenate([gamma.T, beta.T], axis=1).astype(f32),
        "diag": diag.astype(fp8), "dwv": dwv,
    }


_NC = None


def get_nc():
    global _NC
    if _NC is None:
        _NC = _build_nc()
    return _NC


def make_in_maps(inputs):
    x = np.asarray(inputs["x"], np.float32)
    B = x.shape[0]
    prep = prep_inputs(inputs)
    return [{"x": np.ascontiguousarray(x[b]), **prep} for b in range(B)]


def kernel(**inputs):
    nc = get_nc()
    in_maps = make_in_maps(inputs)
    res = bass_utils.run_bass_kernel_spmd(nc, in_maps, core_ids=list(range(8)))
    out = np.stack([res.results[b]["out"] for b in range(8)], axis=0)
    return out.astype(np.float32)


# revision 33
# speedup vs baseline: 1.1328x; 1.1328x over previous
"""Self-contained TRN2 kernel for nn_FLASH_ShareA_FFConvM_FlashAttn.

kernel(**inputs) takes the full (unsharded) inputs from setup_inputs() and
returns the full (B, N, D) float32 output. Internally: data-parallel over the
batch — one batch sample per NeuronCore, 8 cores, no collectives.

v2: all heavy matmuls in fp8 DoubleRow (paired k-tiles / paired conv taps),
depthwise convs fully on the PE as paired diagonal matmuls, attention weights
pre-scaled by 2^30 to stay in fp8 range, deferred output LayerNorm, and the
zspill round-trip replaced by an SBUF-resident transposed buffer.
"""
import sys

if "/opt/trn_rl_repo" not in sys.path:
    sys.path.insert(0, "/opt/trn_rl_repo")

import numpy as np
import ml_dtypes
import concourse.bass as bass
import concourse.bacc as bacc
import concourse.mybir as mybir
import concourse.tile as tile
from concourse import bass_utils
from concourse.ap import AP

F32 = mybir.dt.float32
BF16 = mybir.dt.bfloat16
FP8 = mybir.dt.float8e4
AF = mybir.ActivationFunctionType
OP = mybir.AluOpType
DR = mybir.MatmulPerfMode.DoubleRow

N, D, H, QK, G = 4096, 512, 2048, 128, 256
NG = N // G
NT = N // 128
KTAPS = 17
PAD = 8
NPADBUF = N + 2 * PAD  # fp8/bf16 padded conv input length (max tap read = N+15)
E2 = 2 * D
EPS = 1e-5
NCH = 21  # conv channel tiles: hid 0..15, out 16..19, qk 20
# conv tap pairs with stride-4 spacing (DR rows must be >=4 fp8 elements apart).
# Taps 8,12,9,13,16 run on the DVE instead (frees PE passes during P3/P5).
PAIRS = [(0, 4), (1, 5), (2, 6), (3, 7), (10, 14), (11, 15)]
DVETAPS = [8, 12, 9, 13, 16]
ASCALE = float(2 ** 30)      # attention-weight scale kept inside psum
RELUSC = float(2 ** 15)      # sqrt(ASCALE), applied before squaring
GRP = 16                     # P4 deferred-LN batch size (iterations)


def _pair_ap(t, off, n):
    """Overlapping [128, 2, n] moving AP: row j reads t[:, off+4j : off+4j+n]."""
    base = t[:, 0:1]
    return AP(base.tensor, base.offset + off, [list(base.ap[0]), [4, 2], [1, n]])


def _emit_conv(nc, pool, dvp, dg, wv, h8t, hb, acc):
    """acc = h + conv(h): 6 DR tap pairs + tap16 on PE; 4 taps + identity on DVE."""
    # DVE partial: accd = h + sum_{k in DVETAPS} w_k * h_shift_k   (full width)
    accd = dvp.tile([128, N], BF16, tag="accd")
    tmp = dvp.tile([128, N], BF16, tag="dvtmp")
    for j, k in enumerate(DVETAPS):
        nc.vector.tensor_scalar(out=tmp[:, :], in0=hb[:, k:k + N],
                                scalar1=wv[:, j:j + 1], scalar2=None, op0=OP.mult)
        if j == 0:
            nc.vector.tensor_tensor(out=accd[:, :], in0=tmp[:, :],
                                    in1=hb[:, PAD:PAD + N], op=OP.add)
        else:
            nc.vector.tensor_tensor(out=accd[:, :], in0=tmp[:, :],
                                    in1=accd[:, :], op=OP.add)
    for c in range(8):
        cb = c * 512
        cp = pool.tile([128, 512], F32, tag="convps")
        for pr in range(6):
            nc.tensor.matmul(cp[:, :], dg[:, pr, :, :], _pair_ap(h8t, PAIRS[pr][0] + cb, 512),
                             start=(pr == 0), stop=(pr == 5), perf_mode=DR)
        evac = dvp.tile([128, 512], BF16, tag="cevac")
        nc.scalar.activation(evac[:, :], cp[:, :], AF.Copy)
        nc.vector.tensor_tensor(out=acc[:, cb:cb + 512], in0=evac[:, :],
                                in1=accd[:, cb:cb + 512], op=OP.add)


def _emit(nc, tc, x, wh8, wqk8, wo8, bh, bqk, bo, gb, diag, dwv, out, spill):
    from contextlib import ExitStack
    es = ExitStack()
    consts = es.enter_context(tc.tile_pool(name="consts", bufs=1))
    wh_sb = consts.tile([128, 2, 2, H], FP8)
    nc.sync.dma_start(wh_sb[:, :, :, :], wh8.ap())
    wqk_sb = consts.tile([128, 2, 2, QK], FP8)
    nc.sync.dma_start(wqk_sb[:, :, :, :], wqk8.ap())
    wo_sb = consts.tile([128, 4, 2, D], FP8)
    nc.sync.dma_start(wo_sb[:, :, :, :], wo8.ap())
    bh_sb = consts.tile([128, 16], F32)
    nc.sync.dma_start(bh_sb[:, :], bh.ap())
    bqk_sb = consts.tile([128, 1], F32)
    nc.sync.dma_start(bqk_sb[:, :], bqk.ap())
    bo_sb = consts.tile([128, 4], F32)
    nc.sync.dma_start(bo_sb[:, :], bo.ap())
    gb_sb = consts.tile([128, 8], F32)
    nc.sync.dma_start(gb_sb[:, :], gb.ap())
    dwv_sb = consts.tile([128, NCH, 5], F32)
    nc.sync.dma_start(dwv_sb[:, :, :], dwv.ap())
    eps_sb = consts.tile([128, 1], F32)
    nc.vector.memset(eps_sb[:, :], EPS)

    outer = es.enter_context(tc.tile_pool(name="outer", bufs=1))
    attnT8 = outer.tile([128, NG, 2, G], FP8)
    lq_sb = outer.tile([128, N], BF16)
    lk_str = outer.tile([128, NT, 128], BF16)
    linkv_sb = outer.tile([128, E2], BF16)
    linku_sb = outer.tile([128, E2], BF16)
    sums = outer.tile([128, 32], F32)
    sumsq = outer.tile([128, 32], F32)
    spill_v = spill.ap().rearrange("(tt p) (q c4) -> p tt q c4", p=128, c4=512)
    xs_v = x.ap().rearrange("(c p) d -> p c d", p=128)

    es2 = ExitStack()
    zpool = es2.enter_context(tc.tile_pool(name="zpool", bufs=1))
    qq_sb = zpool.tile([128, N], BF16)
    qkk_sb = zpool.tile([128, N], BF16)
    zT8 = []
    for c in range(8):
        zT8c = zpool.tile([128, 4, 512], FP8, tag=f"zT8_{c}")
        zT8.append(zT8c)

    # ---------------- P0: token-shifted LayerNorm -> zT8 chunks ----------------
    # x loads batched 4 token-tiles per DMA; shifted first-half channels loaded
    # separately with a one-row offset.
    xs_v = x.ap().rearrange("(c p) d -> p c d", p=128)
    with tc.tile_pool(name="p0", bufs=4) as p0, \
         tc.tile_pool(name="p0z", bufs=4) as p0z, \
         tc.tile_pool(name="p0s", bufs=8) as p0s:
        for cch in (0, 1, 2, 3, 4, 5, 6, 7):
            x4 = p0.tile([128, 4, D], F32, tag="x4")
            t0 = cch * 512
            if cch == 0:
                nc.vector.memset(x4[0:1, 0, 0:D // 2], 0.0)
                nc.gpsimd.dma_start(x4[1:128, 0, 0:D // 2], x[0:127, 0:D // 2])
                for j in range(1, 4):
                    nc.gpsimd.dma_start(x4[:, j, 0:D // 2],
                                        x[t0 + j * 128 - 1:t0 + j * 128 + 127, 0:D // 2])
            else:
                nc.gpsimd.dma_start(
                    x4[:, :, 0:D // 2],
                    x.ap()[t0 - 1:t0 + 511, 0:D // 2].rearrange("(j p) d -> p j d", p=128))
            nc.gpsimd.dma_start(x4[:, :, D // 2:D],
                                xs_v[:, 4 * cch:4 * cch + 4, D // 2:D])
            ztc = p0z.tile([128, 4, 512], BF16, tag="ztc")
            for j in range(4):
                st6 = p0s.tile([128, 6], F32, tag="st6")
                nc.vector.bn_stats(st6[:, :], x4[:, j, :])
                mv = p0s.tile([128, 2], F32, tag="mv")
                nc.vector.bn_aggr(mv[:, :], st6[:, :])
                rstd = p0s.tile([128, 1], F32, tag="rstd")
                nc.scalar.activation(rstd[:, :], mv[:, 1:2], AF.Sqrt, bias=eps_sb[:, :], scale=1.0)
                nc.vector.reciprocal(rstd[:, :], rstd[:, :])
                nmu = p0s.tile([128, 1], F32, tag="nmu")
                nc.vector.tensor_scalar(out=nmu[:, :], in0=mv[:, 0:1], scalar1=rstd[:, :],
                                        scalar2=-1.0, op0=OP.mult, op1=OP.mult)
                zt = p0.tile([128, D], BF16, tag="zt")
                nc.scalar.activation(zt[:, :], x4[:, j, :], AF.Identity,
                                     bias=nmu[:, :], scale=rstd[:, :])
                eng = nc.sync if j % 2 == 0 else nc.scalar
                eng.dma_start_transpose(ztc[:, :, j * 128:j * 128 + 128], zt[:, :])
                nc.scalar.activation(zT8[cch][:, :, j * 128:j * 128 + 128],
                                     ztc[:, :, j * 128:j * 128 + 128], AF.Copy)

    # ---------------- P1+P3: qk path interleaved with hidden FFConvM ----------------
    with tc.tile_pool(name="p1", bufs=1) as p1, \
         tc.tile_pool(name="p3", bufs=2) as p3, \
         tc.tile_pool(name="p3d", bufs=3) as p3d, \
         tc.tile_pool(name="p1s", bufs=3) as p1s, \
         tc.tile_pool(name="p3p", bufs=3, space="PSUM") as p3p, \
         tc.tile_pool(name="p3cp", bufs=3, space="PSUM") as p3cp, \
         tc.tile_pool(name="p1sp", bufs=1, space="PSUM") as p1sp, \
         tc.tile_pool(name="p3lin", bufs=1, space="PSUM") as p3lin:
        state = {"strips4": None}

        def produce_start(hc):
            dg = p3d.tile([128, 7, 2, 128], FP8, tag="dg")
            nc.gpsimd.dma_start(dg[:, :, :, :], diag.ap()[:, hc, :, :, :])
            hb = p3.tile([128, NPADBUF], BF16, tag="hpad")
            nc.vector.memset(hb[:, 0:PAD], 0.0)
            nc.vector.memset(hb[:, PAD + N:], 0.0)
            return dg, hb

        def produce_chunk(hc, hb, c):
            ps = p3p.tile([128, 512], F32, tag="hps")
            for pr in range(2):
                nc.tensor.matmul(ps[:, :], wh_sb[:, pr, :, hc * 128:(hc + 1) * 128],
                                 zT8[c][:, 2 * pr:2 * pr + 2, :],
                                 start=(pr == 0), stop=(pr == 1), perf_mode=DR)
            nc.scalar.activation(hb[:, PAD + c * 512:PAD + (c + 1) * 512], ps[:, :],
                                 AF.Silu, bias=bh_sb[:, hc:hc + 1], scale=1.0)

        def produce_end(hb):
            h8 = p3.tile([128, NPADBUF], FP8, tag="h8pad")
            nc.scalar.activation(h8[:, :], hb[:, :], AF.Copy)
            return h8

        def produce(hc):
            dg, hb = produce_start(hc)
            for c in range(8):
                produce_chunk(hc, hb, c)
            return dg, hb, produce_end(hb)

        def convpost(hc, dg, hb, h8):
            if hc % 4 == 0:
                s4_new = p3q.tile([128, NT, 4, 128], BF16, tag="strips4")
                state["strips4"] = s4_new
            strips4 = state["strips4"]
            acc = p3.tile([128, N], BF16, tag="acc")
            _emit_conv(nc, p3cp, p3v, dg, dwv_sb[:, hc, :], h8, hb, acc)
            nc.sync.dma_start_transpose(strips4[:, :, hc % 4, :], acc[:, :])
            if hc % 4 == 3:
                q = hc // 4
                kvp = p3lin.tile([128, 512], F32, tag="kvps")
                for tt in range(NT):
                    nc.tensor.matmul(
                        kvp[:, :], lk_str[:, tt, :],
                        strips4[:, tt, :, :].rearrange("p a c -> p (a c)"),
                        start=(tt == 0), stop=(tt == NT - 1))
                nc.gpsimd.dma_start(spill_v[:, :, q, :], strips4[:, :, :, :])
                dst = linkv_sb if q < 2 else linku_sb
                nc.scalar.activation(dst[:, (q % 2) * 512:(q % 2) * 512 + 512],
                                     kvp[:, :], AF.Copy)

        # Front: chunk-major over {qk, hc0, hc1} so the PE consumes zT8 chunks
        # the moment P0 produces them (no head-of-line blocking on later chunks).
        dgq = p3d.tile([128, 7, 2, 128], FP8, tag="dg")
        nc.gpsimd.dma_start(dgq[:, :, :, :], diag.ap()[:, 20, :, :, :])
        qkp = p1.tile([128, NPADBUF], BF16, tag="qkpad")
        nc.vector.memset(qkp[:, 0:PAD], 0.0)
        nc.vector.memset(qkp[:, PAD + N:], 0.0)
        q8p = p1.tile([128, NPADBUF], FP8, tag="qk8pad")
        dg0, hb0 = produce_start(0)
        dg1, hb1 = produce_start(1)
        for c in range(8):
            ps = p3p.tile([128, 512], F32, tag="hps")
            for pr in range(2):
                nc.tensor.matmul(ps[:, :], wqk_sb[:, pr, :, :],
                                 zT8[c][:, 2 * pr:2 * pr + 2, :],
                                 start=(pr == 0), stop=(pr == 1), perf_mode=DR)
            nc.scalar.activation(qkp[:, PAD + c * 512:PAD + (c + 1) * 512], ps[:, :],
                                 AF.Silu, bias=bqk_sb[:, :], scale=1.0)
            produce_chunk(0, hb0, c)
            produce_chunk(1, hb1, c)
        es4 = ExitStack()
        p3q = es4.enter_context(tc.tile_pool(name="p3q", bufs=1))
        p3v = es4.enter_context(tc.tile_pool(name="p3v", bufs=2))
        nc.scalar.activation(q8p[:, :], qkp[:, :], AF.Copy)
        h80 = produce_end(hb0)
        h81 = produce_end(hb1)
        qkc = p3.tile([128, N], BF16, tag="acc")
        _emit_conv(nc, p3cp, p3v, dgq, dwv_sb[:, 20, :], q8p, qkp, qkc)
        lkk = p3.tile([128, N], BF16, tag="acc")
        for i, dst in ((0, qq_sb), (1, lq_sb), (2, qkk_sb), (3, lkk)):
            nc.vector.tensor_scalar(out=dst[:, :], in0=qkc[:, :], scalar1=gb_sb[:, i:i + 1],
                                    scalar2=gb_sb[:, 4 + i:5 + i], op0=OP.mult, op1=OP.add)
        nc.sync.dma_start_transpose(lk_str[:, :, :], lkk[:, :])
        convpost(0, dg0, hb0, h80)
        for g in range(NG):
            for jh in range(2):
                sp = p1sp.tile([128, G], F32, tag="simps")
                nc.tensor.matmul(sp[:, :],
                                 qkk_sb[:, g * G + jh * 128: g * G + jh * 128 + 128],
                                 qq_sb[:, g * G:(g + 1) * G],
                                 start=True, stop=True)
                rel = p1s.tile([128, G], BF16, tag="rel")
                nc.scalar.activation(rel[:, :], sp[:, :], AF.Relu, scale=RELUSC)
                nc.scalar.activation(attnT8[:, g, jh, :], rel[:, :], AF.Square)

        prev = (1, dg1, hb1, h81)
        for hc in range(2, 16):
            pr = produce(hc)
            convpost(*prev)
            prev = (hc, *pr)
        convpost(*prev)
        es4.close()

    es2.close()  # frees zT8 chunks before the P4/P5 pools open

    # ---------------- P4: attention apply + gating (deferred LN) ----------------
    with tc.tile_pool(name="mid", bufs=1) as mid:
        zoT8 = mid.tile([128, 8, N], FP8)
        vo_big = mid.tile([128, NT, 4, 128], BF16)
        with tc.tile_pool(name="p4", bufs=2) as p4, \
             tc.tile_pool(name="p4g", bufs=GRP + 2) as p4g, \
             tc.tile_pool(name="p4s", bufs=3) as p4s, \
             tc.tile_pool(name="p4p", bufs=4, space="PSUM") as p4p:
            govu = []   # (go, vt?, ...) per pending it in current group
            for g in range(NG):
                vt, ut = [], []
                for jh in range(2):
                    vtj = p4.tile([128, E2], BF16, tag=f"vg{jh}")
                    nc.gpsimd.dma_start(vtj[:, :], spill[g * G + jh * 128: g * G + jh * 128 + 128, 0:E2])
                    utj = p4.tile([128, E2], BF16, tag=f"ug{jh}")
                    nc.gpsimd.dma_start(utj[:, :], spill[g * G + jh * 128: g * G + jh * 128 + 128, E2:H])
                    vt.append(vtj)
                    ut.append(utj)
                vt8 = p4.tile([128, 2, 2, 512], FP8, tag="vt8")
                ut8 = p4.tile([128, 2, 2, 512], FP8, tag="ut8")
                for jh in range(2):
                    for e in range(2):
                        nc.vector.tensor_copy(vt8[:, e, jh, :], vt[jh][:, e * 512:(e + 1) * 512])
                        nc.scalar.activation(ut8[:, e, jh, :], ut[jh][:, e * 512:(e + 1) * 512],
                                             AF.Copy)
                for it in range(2):
                    idx = g * 2 + it
                    islice = slice(g * G + it * 128, g * G + it * 128 + 128)
                    avp = p4p.tile([128, E2], F32, tag="avps")
                    for e in range(2):
                        nc.tensor.matmul(avp[:, e * 512:(e + 1) * 512],
                                         attnT8[:, g, :, it * 128:it * 128 + 128],
                                         vt8[:, e, :, :],
                                         start=True, stop=False, perf_mode=DR)
                        nc.tensor.matmul(avp[:, e * 512:(e + 1) * 512],
                                         lq_sb[:, islice], linkv_sb[:, e * 512:(e + 1) * 512],
                                         start=False, stop=True)
                    t1 = p4s.tile([128, E2], BF16, tag="t1")
                    nc.vector.scalar_tensor_tensor(out=t1[:, :], in0=avp[:, :],
                                                   scalar=1.0 / ASCALE, in1=ut[it][:, :],
                                                   op0=OP.mult, op1=OP.mult)
                    sg = p4s.tile([128, E2], BF16, tag="sg")
                    nc.scalar.activation(sg[:, :], t1[:, :], AF.Sigmoid)
                    aup = p4p.tile([128, E2], F32, tag="avps")
                    for e in range(2):
                        nc.tensor.matmul(aup[:, e * 512:(e + 1) * 512],
                                         attnT8[:, g, :, it * 128:it * 128 + 128],
                                         ut8[:, e, :, :],
                                         start=True, stop=False, perf_mode=DR)
                        nc.tensor.matmul(aup[:, e * 512:(e + 1) * 512],
                                         lq_sb[:, islice], linku_sb[:, e * 512:(e + 1) * 512],
                                         start=False, stop=True)
                    t2 = p4s.tile([128, E2], BF16, tag="t2")
                    nc.vector.scalar_tensor_tensor(out=t2[:, :], in0=aup[:, :],
                                                   scalar=1.0 / ASCALE, in1=vt[it][:, :],
                                                   op0=OP.mult, op1=OP.mult)
                    go = p4g.tile([128, E2], BF16, tag="go")
                    nc.vector.scalar_tensor_tensor(out=go[:, :], in0=t2[:, :], scalar=1.0,
                                                   in1=sg[:, :], op0=OP.mult, op1=OP.mult,
                                                   accum_out=sums[:, idx:idx + 1])
                    jnk = p4s.tile([128, E2], BF16, tag="jnk")
                    nc.scalar.activation(jnk[:, :], go[:, :], AF.Square,
                                         accum_out=sumsq[:, idx:idx + 1])
                    govu.append(go)
                    if len(govu) == GRP:
                        _p4_norm(nc, tc, p4s, govu, sums, sumsq, eps_sb, zoT8,
                                 idx - GRP + 1)
                        govu = []

        # ---------------- P5: output FFConvM ----------------
        with tc.tile_pool(name="p5", bufs=2) as p5, \
             tc.tile_pool(name="p5d", bufs=2) as p5d, \
             tc.tile_pool(name="p5v", bufs=2) as p5v, \
             tc.tile_pool(name="p5p", bufs=2, space="PSUM") as p5p, \
             tc.tile_pool(name="p5cp", bufs=3, space="PSUM") as p5cp:
            def produce5(oc):
                dg = p5d.tile([128, 7, 2, 128], FP8, tag="dg5")
                nc.gpsimd.dma_start(dg[:, :, :, :], diag.ap()[:, 16 + oc, :, :, :])
                hb = p5.tile([128, NPADBUF], BF16, tag="hpad5")
                nc.vector.memset(hb[:, 0:PAD], 0.0)
                nc.vector.memset(hb[:, PAD + N:], 0.0)
                for c in range(8):
                    ps = p5p.tile([128, 512], F32, tag="ops")
                    for pr in range(4):
                        nc.tensor.matmul(ps[:, :], wo_sb[:, pr, :, oc * 128:(oc + 1) * 128],
                                         zoT8[:, 2 * pr:2 * pr + 2, c * 512:(c + 1) * 512],
                                         start=(pr == 0), stop=(pr == 3), perf_mode=DR)
                    nc.scalar.activation(hb[:, PAD + c * 512:PAD + (c + 1) * 512], ps[:, :],
                                         AF.Silu, bias=bo_sb[:, oc:oc + 1], scale=1.0)
                h8 = p5.tile([128, NPADBUF], FP8, tag="h85")
                nc.scalar.activation(h8[:, :], hb[:, :], AF.Copy)
                return dg, hb, h8

            def convpost5(oc, dg, hb, h8):
                acc = p5.tile([128, N], BF16, tag="acc5")
                _emit_conv(nc, p5cp, p5v, dg, dwv_sb[:, 16 + oc, :], h8, hb, acc)
                nc.sync.dma_start_transpose(vo_big[:, :, oc, :], acc[:, :])

            prev = None
            for oc in range(4):
                pr = produce5(oc)
                if prev is not None:
                    convpost5(*prev)
                prev = (oc, *pr)
            convpost5(*prev)

        # ---------------- P6: residual (4 token-tiles per iteration) ----------------
        out_v = out.ap().rearrange("(c p) d -> p c d", p=128)
        with tc.tile_pool(name="p6", bufs=3) as p6:
            for cch in range(8):
                xt = p6.tile([128, 4, D], F32, tag="xt6")
                nc.gpsimd.dma_start(xt[:, :, :], xs_v[:, 4 * cch:4 * cch + 4, :])
                of = p6.tile([128, 4, D], F32, tag="of")
                nc.vector.tensor_tensor(
                    out=of[:, :, :].rearrange("p a c -> p (a c)"),
                    in0=xt[:, :, :].rearrange("p a c -> p (a c)"),
                    in1=vo_big[:, 4 * cch:4 * cch + 4, :, :].rearrange("p a b c -> p (a b c)"),
                    op=OP.add)
                nc.gpsimd.dma_start(out_v[:, 4 * cch:4 * cch + 4, :], of[:, :, :])
    es.close()


def _p4_norm(nc, tc, pool, gos, sums, sumsq, eps_sb, zoT8, idx0):
    """Deferred LayerNorm for GRP gating tiles: batched stats then per-tile
    normalize + transpose + fp8 convert."""
    n = len(gos)
    sl = slice(idx0, idx0 + n)
    mean = pool.tile([128, n], F32, tag="mean")
    nc.vector.tensor_scalar(out=mean[:, :], in0=sums[:, sl], scalar1=1.0 / E2,
                            scalar2=None, op0=OP.mult)
    msq = pool.tile([128, n], F32, tag="msq")
    nc.vector.tensor_tensor(out=msq[:, :], in0=mean[:, :], in1=mean[:, :], op=OP.mult)
    var = pool.tile([128, n], F32, tag="var")
    nc.vector.scalar_tensor_tensor(out=var[:, :], in0=sumsq[:, sl], scalar=1.0 / E2,
                                   in1=msq[:, :], op0=OP.mult, op1=OP.subtract)
    rstd = pool.tile([128, n], F32, tag="rstdn")
    nc.scalar.activation(rstd[:, :], var[:, :], AF.Sqrt, bias=eps_sb[:, :], scale=1.0)
    nc.vector.reciprocal(rstd[:, :], rstd[:, :])
    nmu = pool.tile([128, n], F32, tag="nmun")
    nc.vector.tensor_tensor(out=nmu[:, :], in0=mean[:, :], in1=rstd[:, :], op=OP.mult)
    nc.vector.tensor_scalar(out=nmu[:, :], in0=nmu[:, :], scalar1=-1.0,
                            scalar2=None, op0=OP.mult)
    for j, go in enumerate(gos):
        tti = idx0 + j
        zo = pool.tile([128, E2], BF16, tag="zon")
        nc.vector.tensor_scalar(out=zo[:, :], in0=go[:, :], scalar1=rstd[:, j:j + 1],
                                scalar2=nmu[:, j:j + 1], op0=OP.mult, op1=OP.add)
        zot = pool.tile([128, 8, 128], BF16, tag="zot")
        (nc.sync if j % 2 == 0 else nc.scalar).dma_start_transpose(zot[:, :, :], zo[:, :])
        if j % 2 == 0:
            nc.scalar.activation(zoT8[:, :, tti * 128:(tti + 1) * 128], zot[:, :, :], AF.Copy)
        else:
            nc.vector.tensor_copy(zoT8[:, :, tti * 128:(tti + 1) * 128], zot[:, :, :])


def _build_nc():
    nc = bacc.Bacc("TRN2", target_bir_lowering=False, debug=False)
    x = nc.dram_tensor("x", [N, D], F32, kind="ExternalInput")
    wh8 = nc.dram_tensor("wh8", [128, 2, 2, H], FP8, kind="ExternalInput")
    wqk8 = nc.dram_tensor("wqk8", [128, 2, 2, QK], FP8, kind="ExternalInput")
    wo8 = nc.dram_tensor("wo8", [128, 4, 2, D], FP8, kind="ExternalInput")
    bh = nc.dram_tensor("bh", [128, 16], F32, kind="ExternalInput")
    bqk = nc.dram_tensor("bqk", [128, 1], F32, kind="ExternalInput")
    bo = nc.dram_tensor("bo", [128, 4], F32, kind="ExternalInput")
    gb = nc.dram_tensor("gb", [128, 8], F32, kind="ExternalInput")
    diag = nc.dram_tensor("diag", [128, NCH, 7, 2, 128], FP8, kind="ExternalInput")
    dwv = nc.dram_tensor("dwv", [128, NCH, 5], F32, kind="ExternalInput")
    out = nc.dram_tensor("out", [N, D], F32, kind="ExternalOutput")
    spill = nc.dram_tensor("spill", [N, H], BF16)
    with tile.TileContext(nc) as tc:
        _emit(nc, tc, x, wh8, wqk8, wo8, bh, bqk, bo, gb, diag, dwv, out, spill)
    nc.compile()
    return nc


def prep_inputs(inputs):
    f32 = np.float32
    fp8 = ml_dtypes.float8_e4m3
    W_h = np.asarray(inputs["W_h"], f32)
    W_qk = np.asarray(inputs["W_qk"], f32)
    W_o = np.asarray(inputs["W_o"], f32)
    whp = np.asarray(inputs["ln_h_g"], f32)[:, None] * W_h
    bhp = np.asarray(inputs["ln_h_b"], f32) @ W_h + np.asarray(inputs["b_h"], f32)
    wqkp = np.asarray(inputs["ln_qk_g"], f32)[:, None] * W_qk
    bqkp = np.asarray(inputs["ln_qk_b"], f32) @ W_qk + np.asarray(inputs["b_qk"], f32)
    wop = np.asarray(inputs["ln_o_g"], f32)[:, None] * W_o
    bop = np.asarray(inputs["ln_o_b"], f32) @ W_o + np.asarray(inputs["b_o"], f32)
    gamma = np.asarray(inputs["gamma"], f32).copy()
    beta = np.asarray(inputs["beta"], f32).copy()
    gamma[0] /= G
    beta[0] /= G
    gamma[1] *= ASCALE
    beta[1] *= ASCALE
    gamma[3] /= N
    beta[3] /= N

    def lhsT8(w, kt):
        # [din, dout] -> [128, kt/2 pairs, 2, dout] fp8
        t = w.reshape(kt, 128, -1).transpose(1, 0, 2)  # [128, kt, dout]
        return np.ascontiguousarray(
            t.reshape(128, kt // 2, 2, t.shape[-1])).astype(fp8)

    def chan(v, ntiles):
        return np.ascontiguousarray(v.reshape(ntiles, 128).T).astype(f32)

    # diagonal conv stationaries: [128, NCH, 9, 2, 128] fp8
    dw_h = np.asarray(inputs["dw_h"], f32)
    dw_o = np.asarray(inputs["dw_o"], f32)
    dw_qk = np.asarray(inputs["dw_qk"], f32)
    diag = np.zeros((128, NCH, 7, 2, 128), f32)
    dwv = np.zeros((128, NCH, 5), f32)
    rng128 = np.arange(128)
    for ct in range(NCH):
        if ct < 16:
            wsrc = dw_h[:, ct * 128:(ct + 1) * 128]
        elif ct < 20:
            wsrc = dw_o[:, (ct - 16) * 128:(ct - 15) * 128]
        else:
            wsrc = dw_qk
        for pr, (k0, k1) in enumerate(PAIRS):
            diag[rng128, ct, pr, 0, rng128] = wsrc[k0]
            diag[rng128, ct, pr, 1, rng128] = wsrc[k1]
        for j, k in enumerate(DVETAPS):
            dwv[:, ct, j] = wsrc[k]
    return {
        "wh8": lhsT8(whp, 4), "wqk8": lhsT8(wqkp, 4), "wo8": lhsT8(wop, 8),
        "bh": chan(bhp, 16), "bqk": chan(bqkp, 1), "bo": chan(bop, 4),
        "gb": np.concatenate([gamma.T, beta.T], axis=1).astype(f32),
        "diag": diag.astype(fp8), "dwv": dwv,
    }


_NC = None


def get_nc():
    global _NC
    if _NC is None:
        _NC = _build_nc()
    return _NC


def make_in_maps(inputs):
    x = np.asarray(inputs["x"], np.float32)
    B = x.shape[0]
    prep = prep_inputs(inputs)
    return [{"x": np.ascontiguousarray(x[b]), **prep} for b in range(B)]


def kernel(**inputs):
    nc = get_nc()
    in_maps = make_in_maps(inputs)
    res = bass_utils.run_bass_kernel_spmd(nc, in_maps, core_ids=list(range(8)))
    out = np.stack([res.results[b]["out"] for b in range(8)], axis=0)
    return out.astype(np.float32)
